# revision 17
# baseline (speedup 1.0000x reference)
"""Embedding gather-sum kernel for Trainium2 (8 NeuronCores, SPMD).

Problem (nn_UserLinearUpscaler):
    out[b, s, :] = sum_k W[:, ids[b, s, k]] + bias
    B=1024, S=50, K=20, E=64, V=100000, f32 weights, integer ids.

Sharding: data-parallel over batch — each of the 8 cores handles 128 batch
rows (6400 tokens = 128000 row lookups) with W.T ([V, 64] f32) replicated
per core in DRAM.

Per core, per chunk of 320 tokens (= 6400 (token, k) lookup slots):
  1. Host splits the chunk's ids into 4 vocab ranges of 32768 rows
     (range = id >> 15, local = id & 32767 — int16-safe for dma_gather)
     as compact index lists, padded to static sizes.  num_idxs_reg is the
     max true count over the 8 cores, so the static padding tail costs no
     descriptors (trailing -1 entries are skipped by the DGE).
  2. nc.gpsimd.dma_gather fetches the rows (256 B each) from W.T into
     compact SBUF tiles (slot i -> partition i%128, column i//128).
  3. For each 128-slot block, VectorE builds a [128, 320] 0/1 selection
     matrix S = is_equal(token_of_slot, iota) and the TensorEngine
     accumulates   psum[64, 320] += cg_block[128, 64].T @ S[128, 320].
     Padding slots carry token -1 and select nothing.  PSUM ends up with
     the chunk's output transposed (embedding dim on partitions).
  4. Bias (a [64, 1] column) is added on PSUM eviction; the [64, 320] tile
     is DMA'd out and the host transposes back at the end.

The kernel is descriptor-generation bound: the Q7 SWDGE emits one DMA
descriptor per gathered row at ~9 ns/row, so everything else (HBM traffic,
S-builds, matmuls) hides underneath.  single_packet=False is required for
gathers above ~2.5k indices.
"""

import numpy as np

import concourse.bass as bass
import concourse.tile as tile
from concourse import bacc, mybir
from concourse.bass_utils import run_bass_kernel_spmd

B, S, K, E, V = 1024, 50, 20, 64, 100000
N_CORES = 8
P = 128
TOK_CORE = B // N_CORES * S          # 6400 tokens per core

T3 = 320                             # tokens per chunk (= PSUM window)
CH3 = TOK_CORE // T3                 # 20 chunks
SLOTS3 = T3 * K                      # 6400 lookup slots per chunk

RANGE_BASES = [0, 32768, 65536, 98304]
RANGE_SIZES = [32768, 32768, 32768, V - 98304]
# static padded per-range list lengths (multiples of 128); binomial mean for
# ranges 0-2 is 6400*32768/100000 = 2097 (sigma ~38), range 3 mean 109
N_SLOTS3 = (2304, 2304, 2304, 256)

DMA_SCRATCH = 32768                  # SWDGE descriptor-ring carveout (bytes)

_cache: dict = {}


def _build_v3(n_slots=N_SLOTS3, n_repeat=1, reg_counts=None):
    nblk = tuple(n // P for n in n_slots)
    nblk_tot = sum(nblk)
    nw16 = sum(n_slots) // 16
    nc = bacc.Bacc("TRN2", target_bir_lowering=False, debug=False,
                   num_devices=N_CORES,
                   dynamic_dma_scratch_size=DMA_SCRATCH)
    wt = nc.dram_tensor("wt", [V, E], mybir.dt.float32, kind="ExternalInput")
    gidx = nc.dram_tensor("gidx", [CH3, P, nw16], mybir.dt.int16,
                          kind="ExternalInput")
    tokf = nc.dram_tensor("tokf", [CH3, P, nblk_tot], mybir.dt.float32,
                          kind="ExternalInput")
    iota = nc.dram_tensor("iota", [P, T3], mybir.dt.float32,
                          kind="ExternalInput")
    biasc = nc.dram_tensor("biasc", [E, 1], mybir.dt.float32,
                           kind="ExternalInput")
    y = nc.dram_tensor("y", [CH3, E, T3], mybir.dt.float32,
                       kind="ExternalOutput")

    with tile.TileContext(nc) as tc:
        with (
            tc.tile_pool(name="idxp", bufs=2) as idxp,
            tc.tile_pool(name="constp", bufs=1) as constp,
            tc.tile_pool(name="cgp", bufs=2) as cgp,
            tc.tile_pool(name="sp", bufs=6) as sp,
            tc.tile_pool(name="psump", bufs=2, space="PSUM") as psump,
            tc.tile_pool(name="evp", bufs=2) as evp,
        ):
            iota_t = constp.tile([P, T3], mybir.dt.float32)
            nc.sync.dma_start(out=iota_t[:, :], in_=iota[:, :])
            biasc_t = constp.tile([E, 1], mybir.dt.float32)
            nc.sync.dma_start(out=biasc_t[:, :], in_=biasc[:, :])

            for _ in range(n_repeat):
                for c in range(CH3):
                    gidx_t = idxp.tile([P, nw16], mybir.dt.int16, tag="gidx")
                    nc.sync.dma_start(out=gidx_t[:, :], in_=gidx[c])
                    tokf_t = idxp.tile([P, nblk_tot], mybir.dt.float32,
                                       tag="tokf")
                    nc.sync.dma_start(out=tokf_t[:, :], in_=tokf[c])

                    cgs = []
                    off = 0
                    for r in range(4):
                        n_r = n_slots[r]
                        cg = cgp.tile([P, n_r // P, E], mybir.dt.float32,
                                      tag=f"cg{r}")
                        n_used = (reg_counts[c][r]
                                  if reg_counts is not None else n_r)
                        nc.gpsimd.dma_gather(
                            out_ap=cg[:, :, :],
                            in_ap=wt[RANGE_BASES[r]:
                                     RANGE_BASES[r] + RANGE_SIZES[r], :],
                            idxs_ap=gidx_t[:, off:off + n_r // 16],
                            num_idxs=n_r,
                            num_idxs_reg=n_used,
                            elem_size=E,
                            single_packet=False,
                        )
                        cgs.append(cg)
                        off += n_r // 16

                    psum = psump.tile([E, T3], mybir.dt.float32, tag="ps")
                    blk = 0
                    for r in range(4):
                        for bb in range(nblk[r]):
                            s_t = sp.tile([P, T3], mybir.dt.float32, tag="S")
                            nc.vector.tensor_tensor(
                                out=s_t[:, :],
                                in0=tokf_t[:, blk:blk + 1].to_broadcast([P, T3]),
                                in1=iota_t[:, :],
                                op=mybir.AluOpType.is_equal)
                            nc.tensor.matmul(
                                out=psum[:, :],
                                lhsT=cgs[r][:, bb, :],
                                rhs=s_t[:, :],
                                start=(blk == 0),
                                stop=(blk == nblk_tot - 1))
                            blk += 1

                    ev = evp.tile([E, T3], mybir.dt.float32, tag="ev")
                    nc.vector.tensor_tensor(
                        out=ev[:, :], in0=psum[:, :],
                        in1=biasc_t[:, 0:1].to_broadcast([E, T3]),
                        op=mybir.AluOpType.add)
                    nc.sync.dma_start(out=y[c], in_=ev[:, :])
    nc.compile()
    return nc


def _wrap16(flat: np.ndarray) -> np.ndarray:
    """int16 list -> [128, n/16] layout (index i at partition i%16, column
    i//16, replicated across the 8 16-partition Q7 groups)."""
    n = flat.shape[0]
    blk = flat.reshape(n // 16, 16).T            # [16, n/16]
    return np.tile(blk, (8, 1))


def _build_indices_v3(ids_core: np.ndarray, n_slots,
                      reg_counts=None) -> tuple[np.ndarray, np.ndarray]:
    """ids_core: [TOK_CORE, K] int32 -> (gidx [CH3, P, nw16] int16,
    tokf [CH3, P, nblk_tot] f32)."""
    nblk = tuple(n // P for n in n_slots)
    nblk_tot = sum(nblk)
    nw16 = sum(n_slots) // 16
    gidx = np.zeros((CH3, P, nw16), np.int16)
    tokf = np.zeros((CH3, P, nblk_tot), np.float32)
    tok_of_slot = np.arange(SLOTS3) // K
    for c in range(CH3):
        flat = ids_core[c * T3:(c + 1) * T3].reshape(-1)      # [SLOTS3]
        rng_id = flat >> 15
        local = flat & 32767
        off = 0
        boff = 0
        for r in range(4):
            sel = np.nonzero(rng_id == r)[0]
            n_r = n_slots[r]
            if sel.shape[0] > n_r:
                raise OverflowError(f"range {r}: {sel.shape[0]} > {n_r}")
            n_used = reg_counts[c][r] if reg_counts is not None else n_r
            g = np.full(n_r, -1, np.int16)     # tail skipped by descgen
            g[:n_used] = 0                      # filler up to shared count
            g[:sel.shape[0]] = local[sel]
            tf = np.full(n_r, -1.0, np.float32)  # pads select no token
            tf[:sel.shape[0]] = tok_of_slot[sel]
            gidx[c, :, off:off + n_r // 16] = _wrap16(g)
            tokf[c, :, boff:boff + nblk[r]] = tf.reshape(nblk[r], P).T
            off += n_r // 16
            boff += nblk[r]
    return gidx, tokf


def kernel(content_input: np.ndarray, W: np.ndarray, b: np.ndarray) -> np.ndarray:
    ids = np.ascontiguousarray(content_input).astype(np.int32).reshape(B * S, K)
    wt = np.ascontiguousarray(W.T.astype(np.float32))
    iota = np.ascontiguousarray(
        np.broadcast_to(np.arange(T3, dtype=np.float32), (P, T3)))
    biasc = np.ascontiguousarray(b.astype(np.float32).reshape(E, 1))

    per_core = [ids[i * TOK_CORE:(i + 1) * TOK_CORE] for i in range(N_CORES)]
    # per-(chunk, range) max count across cores -> num_idxs_reg constants;
    # static list sizes grow (one recompile) only if an input distribution
    # overflows the default padding
    cnt = np.zeros((CH3, 4), np.int64)
    for pc in per_core:
        for c in range(CH3):
            r = pc[c * T3:(c + 1) * T3].reshape(-1) >> 15
            cnt[c] = np.maximum(cnt[c], np.bincount(r, minlength=4))
    n_slots = tuple(
        int(max(d, -(-int(m) // P) * P))
        for d, m in zip(N_SLOTS3, cnt.max(axis=0)))
    reg_counts = tuple(tuple(max(int(v), 16) for v in row) for row in cnt)

    key = ("nc3", n_slots, reg_counts)
    if key not in _cache:
        _cache[key] = _build_v3(n_slots, reg_counts=reg_counts)
    nc = _cache[key]

    in_maps = []
    for i in range(N_CORES):
        gidx, tokf = _build_indices_v3(per_core[i], n_slots, reg_counts)
        in_maps.append({"wt": wt, "gidx": gidx, "tokf": tokf,
                        "iota": iota, "biasc": biasc})
    res = run_bass_kernel_spmd(nc, in_maps, core_ids=list(range(N_CORES)))
    # y[c, :, t] holds out[token c*T3 + t, :] transposed
    out = np.concatenate(
        [res.results[i]["y"].transpose(0, 2, 1).reshape(TOK_CORE, E)
         for i in range(N_CORES)],
        axis=0)
    return out.reshape(B, S, E)


# revision 19
# speedup vs baseline: 1.0658x; 1.0658x over previous
"""Embedding gather-sum kernel for Trainium2 (8 NeuronCores, SPMD).

Problem (nn_UserLinearUpscaler):
    out[b, s, :] = sum_k W[:, ids[b, s, k]] + bias
    B=1024, S=50, K=20, E=64, V=100000, f32 weights, integer ids.

Sharding: data-parallel over batch — each of the 8 cores handles 128 batch
rows (6400 tokens = 128000 row lookups) with W.T ([V, 64] f32) replicated
per core in DRAM.

Per core, per chunk of 320 tokens (= 6400 (token, k) lookup slots):
  1. Host splits the chunk's ids into 4 vocab ranges of 32768 rows
     (range = id >> 15, local = id & 32767 — int16-safe for dma_gather)
     as compact index lists, padded to static sizes.  num_idxs_reg is the
     max true count over the 8 cores, so the static padding tail costs no
     descriptors (trailing -1 entries are skipped by the DGE).
  2. nc.gpsimd.dma_gather fetches the rows (256 B each) from W.T into
     compact SBUF tiles (slot i -> partition i%128, column i//128).
  3. For each 128-slot block, VectorE builds a [128, 320] 0/1 selection
     matrix S = is_equal(token_of_slot, iota) and the TensorEngine
     accumulates   psum[64, 320] += cg_block[128, 64].T @ S[128, 320].
     Padding slots carry token -1 and select nothing.  PSUM ends up with
     the chunk's output transposed (embedding dim on partitions).
  4. Bias (a [64, 1] column) is added on PSUM eviction; the [64, 320] tile
     is DMA'd out and the host transposes back at the end.

The kernel is descriptor-generation bound: the Q7 SWDGE emits one DMA
descriptor per gathered row at ~9 ns/row, so everything else (HBM traffic,
S-builds, matmuls) hides underneath.  single_packet=False is required for
gathers above ~2.5k indices.
"""

import numpy as np

import concourse.bass as bass
import concourse.tile as tile
from concourse import bacc, mybir
from concourse.bass_utils import run_bass_kernel_spmd

B, S, K, E, V = 1024, 50, 20, 64, 100000
N_CORES = 8
P = 128
TOK_CORE = B // N_CORES * S          # 6400 tokens per core

T3 = 320                             # tokens per chunk (= PSUM window)
CH3 = TOK_CORE // T3                 # 20 chunks
SLOTS3 = T3 * K                      # 6400 lookup slots per chunk

RANGE_BASES = [0, 32768, 65536, 98304]
RANGE_SIZES = [32768, 32768, 32768, V - 98304]
# static padded per-range list lengths (multiples of 128); binomial mean for
# ranges 0-2 is 6400*32768/100000 = 2097 (sigma ~38), range 3 mean 109
N_SLOTS3 = (2304, 2304, 2304, 256)

DMA_SCRATCH = 32768                  # SWDGE descriptor-ring carveout (bytes)

_cache: dict = {}


def _build_v3(n_slots=N_SLOTS3, n_repeat=1, reg_counts=None):
    nblk = tuple(n // P for n in n_slots)
    nblk_tot = sum(nblk)
    nw16 = sum(n_slots) // 16
    nc = bacc.Bacc("TRN2", target_bir_lowering=False, debug=False,
                   num_devices=N_CORES,
                   dynamic_dma_scratch_size=DMA_SCRATCH)
    wt = nc.dram_tensor("wt", [V, E], mybir.dt.float32, kind="ExternalInput")
    gidx = nc.dram_tensor("gidx", [CH3, P, nw16], mybir.dt.int16,
                          kind="ExternalInput")
    tokf = nc.dram_tensor("tokf", [CH3, P, nblk_tot], mybir.dt.float32,
                          kind="ExternalInput")
    iota = nc.dram_tensor("iota", [P, T3], mybir.dt.float32,
                          kind="ExternalInput")
    biasc = nc.dram_tensor("biasc", [E, 1], mybir.dt.float32,
                           kind="ExternalInput")
    y = nc.dram_tensor("y", [CH3, E, T3], mybir.dt.float32,
                       kind="ExternalOutput")

    with tile.TileContext(nc) as tc:
        with (
            tc.tile_pool(name="idxp", bufs=2) as idxp,
            tc.tile_pool(name="constp", bufs=1) as constp,
            tc.tile_pool(name="cgp", bufs=2) as cgp,
            tc.tile_pool(name="sp", bufs=6) as sp,
            tc.tile_pool(name="psump", bufs=2, space="PSUM") as psump,
            tc.tile_pool(name="evp", bufs=2) as evp,
        ):
            iota_t = constp.tile([P, T3], mybir.dt.float32)
            nc.sync.dma_start(out=iota_t[:, :], in_=iota[:, :])
            biasc_t = constp.tile([E, 1], mybir.dt.float32)
            nc.sync.dma_start(out=biasc_t[:, :], in_=biasc[:, :])

            for _ in range(n_repeat):
                for c in range(CH3):
                    gidx_t = idxp.tile([P, nw16], mybir.dt.int16, tag="gidx")
                    nc.sync.dma_start(out=gidx_t[:, :], in_=gidx[c])
                    tokf_t = idxp.tile([P, nblk_tot], mybir.dt.float32,
                                       tag="tokf")
                    nc.sync.dma_start(out=tokf_t[:, :], in_=tokf[c])

                    cgs = []
                    off = 0
                    for r in range(4):
                        n_r = n_slots[r]
                        cg = cgp.tile([P, n_r // P, E], mybir.dt.float32,
                                      tag=f"cg{r}")
                        n_used = (reg_counts[c][r]
                                  if reg_counts is not None else n_r)
                        nc.gpsimd.dma_gather(
                            out_ap=cg[:, :, :],
                            in_ap=wt[RANGE_BASES[r]:
                                     RANGE_BASES[r] + RANGE_SIZES[r], :],
                            idxs_ap=gidx_t[:, off:off + n_r // 16],
                            num_idxs=n_r,
                            num_idxs_reg=n_used,
                            elem_size=E,
                            single_packet=False,
                        )
                        cgs.append(cg)
                        off += n_r // 16

                    psum = psump.tile([E, T3], mybir.dt.float32, tag="ps")
                    blk = 0
                    for r in range(4):
                        for bb in range(nblk[r]):
                            s_t = sp.tile([P, T3], mybir.dt.float32, tag="S")
                            nc.vector.tensor_tensor(
                                out=s_t[:, :],
                                in0=tokf_t[:, blk:blk + 1].to_broadcast([P, T3]),
                                in1=iota_t[:, :],
                                op=mybir.AluOpType.is_equal)
                            nc.tensor.matmul(
                                out=psum[:, :],
                                lhsT=cgs[r][:, bb, :],
                                rhs=s_t[:, :],
                                start=(blk == 0),
                                stop=(blk == nblk_tot - 1))
                            blk += 1

                    ev = evp.tile([E, T3], mybir.dt.float32, tag="ev")
                    nc.vector.tensor_tensor(
                        out=ev[:, :], in0=psum[:, :],
                        in1=biasc_t[:, 0:1].to_broadcast([E, T3]),
                        op=mybir.AluOpType.add)
                    nc.sync.dma_start(out=y[c], in_=ev[:, :])
    nc.compile()
    return nc


def _wrap16(flat: np.ndarray) -> np.ndarray:
    """int16 list -> [128, n/16] layout (index i at partition i%16, column
    i//16, replicated across the 8 16-partition Q7 groups)."""
    n = flat.shape[0]
    blk = flat.reshape(n // 16, 16).T            # [16, n/16]
    return np.tile(blk, (8, 1))


def _build_indices_v3(ids_core: np.ndarray, n_slots,
                      reg_counts=None) -> tuple[np.ndarray, np.ndarray]:
    """ids_core: [TOK_CORE, K] int32 -> (gidx [CH3, P, nw16] int16,
    tokf [CH3, P, nblk_tot] f32)."""
    nblk = tuple(n // P for n in n_slots)
    nblk_tot = sum(nblk)
    nw16 = sum(n_slots) // 16
    gidx = np.zeros((CH3, P, nw16), np.int16)
    tokf = np.zeros((CH3, P, nblk_tot), np.float32)
    tok_of_slot = np.arange(SLOTS3) // K
    for c in range(CH3):
        flat = ids_core[c * T3:(c + 1) * T3].reshape(-1)      # [SLOTS3]
        rng_id = flat >> 15
        local = flat & 32767
        off = 0
        boff = 0
        for r in range(4):
            sel = np.nonzero(rng_id == r)[0]
            n_r = n_slots[r]
            if sel.shape[0] > n_r:
                raise OverflowError(f"range {r}: {sel.shape[0]} > {n_r}")
            n_used = reg_counts[c][r] if reg_counts is not None else n_r
            g = np.full(n_r, -1, np.int16)     # tail skipped by descgen
            g[:n_used] = 0                      # filler up to shared count
            g[:sel.shape[0]] = local[sel]
            tf = np.full(n_r, -1.0, np.float32)  # pads select no token
            tf[:sel.shape[0]] = tok_of_slot[sel]
            gidx[c, :, off:off + n_r // 16] = _wrap16(g)
            tokf[c, :, boff:boff + nblk[r]] = tf.reshape(nblk[r], P).T
            off += n_r // 16
            boff += nblk[r]
    return gidx, tokf


def kernel(content_input: np.ndarray, W: np.ndarray, b: np.ndarray) -> np.ndarray:
    ids = np.ascontiguousarray(content_input).astype(np.int32).reshape(B * S, K)
    wt = np.ascontiguousarray(W.T.astype(np.float32))
    iota = np.ascontiguousarray(
        np.broadcast_to(np.arange(T3, dtype=np.float32), (P, T3)))
    biasc = np.ascontiguousarray(b.astype(np.float32).reshape(E, 1))

    per_core = [ids[i * TOK_CORE:(i + 1) * TOK_CORE] for i in range(N_CORES)]
    # per-(chunk, range) max count across cores -> num_idxs_reg constants;
    # static list sizes grow (one recompile) only if an input distribution
    # overflows the default padding
    cnt = np.zeros((CH3, 4), np.int64)
    for pc in per_core:
        for c in range(CH3):
            r = pc[c * T3:(c + 1) * T3].reshape(-1) >> 15
            cnt[c] = np.maximum(cnt[c], np.bincount(r, minlength=4))
    n_slots = tuple(
        int(max(d, -(-int(m) // P) * P))
        for d, m in zip(N_SLOTS3, cnt.max(axis=0)))
    reg_counts = tuple(tuple(max(int(v), 16) for v in row) for row in cnt)

    key = ("nc3", n_slots, reg_counts)
    if key not in _cache:
        _cache[key] = _build_v3(n_slots, reg_counts=reg_counts)
    nc = _cache[key]

    in_maps = []
    for i in range(N_CORES):
        gidx, tokf = _build_indices_v3(per_core[i], n_slots, reg_counts)
        in_maps.append({"wt": wt, "gidx": gidx, "tokf": tokf,
                        "iota": iota, "biasc": biasc})
    res = run_bass_kernel_spmd(nc, in_maps, core_ids=list(range(N_CORES)))
    # y[c, :, t] holds out[token c*T3 + t, :] transposed
    out = np.concatenate(
        [res.results[i]["y"].transpose(0, 2, 1).reshape(TOK_CORE, E)
         for i in range(N_CORES)],
        axis=0)
    return out.reshape(B, S, E)
